# revision 3
# baseline (speedup 1.0000x reference)
# Distributed GQA attention prefill kernel for one TRN2 chip (8 NeuronCores).
#
# Problem: B=2, S=1024, D=2048, H=32 q-heads, KV=4 kv-heads, HD=64, causal,
# RoPE, f32 I/O. Sharding: core d = (batch g=d//4, kv-head kv=d%4). Each core
# computes q-proj for its 8 q heads, k/v-proj for its kv head, attention, and
# a partial o_proj ([S, 512] @ wo[512kv:512kv+512, :]). Four bf16
# ReduceScatters (one per q-block pair, rows of s-chunks {j, j+4} stacked)
# over each group of 4 cores sum the partials while later pairs compute.
#
# Compute runs in bf16 (f32 PSUM accumulation); f32 matmul on TRN2 is 4
# cycles/row vs 1 for bf16. All DRAM loads are gpsimd casting DMAs
# (f32 -> bf16 on the fly). Attention processes 2 heads per matmul (they
# share the kv head) to halve instruction count and keep the PE pipelined.
import sys

import numpy as np

try:
    import concourse.bass as bass  # noqa: F401
except ImportError:
    for p in ("/opt/trn_rl_repo", "/root/.axon_site/_ro/trn_rl_repo"):
        if p not in sys.path:
            sys.path.append(p)
    import concourse.bass as bass  # noqa: F401

import concourse.bacc as bacc
import concourse.mybir as mybir
import concourse.tile as tile
from concourse import masks
from concourse.bass_utils import run_bass_kernel_spmd

S = 1024
D = 2048
H = 32
KV = 4
HD = 64
NH = 8  # q heads per core
P = 128
SC = S // P  # 8 seq chunks
DC = D // P  # 16 D chunks
N_CORES = 8
GROUPS = [[0, 1, 2, 3], [4, 5, 6, 7]]

F32 = mybir.dt.float32
BF16 = mybir.dt.bfloat16

_NC_CACHE = {}


def _build_graph():
    nc = bacc.Bacc("TRN2", target_bir_lowering=False, debug=False, num_devices=N_CORES)

    x_p = nc.dram_tensor("x", [S, D], F32, kind="ExternalInput")
    wq_p = nc.dram_tensor("wq", [D, NH * HD], F32, kind="ExternalInput")
    wk_p = nc.dram_tensor("wk", [D, HD], F32, kind="ExternalInput")
    wv_p = nc.dram_tensor("wv", [D, HD], F32, kind="ExternalInput")
    wo_p = nc.dram_tensor("wo", [NH * HD, D], F32, kind="ExternalInput")
    cs_p = nc.dram_tensor("cs8", [S, NH * 32], F32, kind="ExternalInput")
    sn_p = nc.dram_tensor("sn8", [S, NH * 32], F32, kind="ExternalInput")
    mk_p = nc.dram_tensor("mk8", [SC, P, P], F32, kind="ExternalInput")
    out_p = nc.dram_tensor("out", [S // 4, D], F32, kind="ExternalOutput")

    with tile.TileContext(nc) as tc:
        with (
            tc.tile_pool(name="const", bufs=1) as constp,
            tc.tile_pool(name="big", bufs=1) as bigp,
            tc.tile_pool(name="work", bufs=1) as workp,
            tc.tile_pool(name="rt", bufs=4) as rtp,
            tc.tile_pool(name="attn", bufs=3) as attnp,
            tc.tile_pool(name="opart", bufs=2) as opartp,
            tc.tile_pool(name="tiny", bufs=4) as tinyp,
            tc.tile_pool(name="psum", bufs=1, space="PSUM") as psump,
            tc.tile_pool(name="dram", bufs=1, space="DRAM") as dramp,
        ):
            # ---- constants (cheap engine work first; DMAs ordered by need) ----
            ident = constp.tile([P, P], BF16, tag="ident")
            masks.make_identity(nc, ident[:])
            ones64 = constp.tile([1, 64], BF16, tag="ones64")
            nc.any.memset(ones64[:], 1.0)

            # x loads split per s-chunk so transposes can start early
            xt_all = bigp.tile([P, SC * D], BF16, tag="x_all")  # x[128s+p, d] at (p, s, d)
            for s in range(SC):
                nc.gpsimd.dma_start(
                    out=xt_all[:, s * D : (s + 1) * D],
                    in_=x_p[s * P : (s + 1) * P, :],
                )
            wk_all = bigp.tile([P, DC * HD], BF16, tag="wk_all")
            nc.gpsimd.dma_start(
                out=wk_all[:].rearrange("p (c n) -> p c n", c=DC),
                in_=wk_p[:, :].rearrange("(c p) n -> p c n", p=P),
            )
            wv_all = bigp.tile([P, DC * HD], BF16, tag="wv_all")
            nc.gpsimd.dma_start(
                out=wv_all[:].rearrange("p (c n) -> p c n", c=DC),
                in_=wv_p[:, :].rearrange("(c p) n -> p c n", p=P),
            )
            wq_all = bigp.tile([P, DC * 512], BF16, tag="wq_all")
            nc.gpsimd.dma_start(
                out=wq_all[:].rearrange("p (c n) -> p c n", c=DC),
                in_=wq_p[:, :].rearrange("(c p) n -> p c n", p=P),
            )
            cst = constp.tile([P, SC * 256], F32, tag="cst")  # cos, tiled x8 heads
            nc.gpsimd.dma_start(
                out=cst[:].rearrange("p (s c) -> p s c", s=SC),
                in_=cs_p[:, :].rearrange("(s p) c -> p s c", p=P),
            )
            snt = constp.tile([P, SC * 256], F32, tag="snt")
            nc.gpsimd.dma_start(
                out=snt[:].rearrange("p (s c) -> p s c", s=SC),
                in_=sn_p[:, :].rearrange("(s p) c -> p s c", p=P),
            )
            mkt = constp.tile([P, SC * P], F32, tag="mkt")  # 8 transposed diag blocks *8
            nc.gpsimd.dma_start(
                out=mkt[:].rearrange("p (j c) -> p j c", j=SC),
                in_=mk_p[:, :, :].rearrange("j p c -> p j c"),
            )
            wo_all = bigp.tile([P, 4 * D], BF16, tag="wo_all")
            nc.gpsimd.dma_start(
                out=wo_all[:].rearrange("p (c n) -> p c n", c=4),
                in_=wo_p[:, :].rearrange("(c p) n -> p c n", p=P),
            )

            # ---- xT: transpose x to [D, S] layout (bf16, via PE) ----
            xT = [workp.tile([P, S], BF16, tag=f"xT{d}", name=f"xT{d}") for d in range(DC)]
            for s in range(SC):
                for d in range(DC):
                    tp = psump.tile([P, P], BF16, tag="sc", bufs=2)
                    nc.tensor.transpose(
                        tp[:], xt_all[:, s * D + d * P : s * D + (d + 1) * P], ident[:]
                    )
                    nc.any.tensor_copy(xT[d][:, s * P : (s + 1) * P], tp[:])

            # ---- projections + RoPE (natural [s, ch] layout) ----
            qrot = [workp.tile([P, NH * HD], BF16, tag=f"qr{s}", name=f"qr{s}") for s in range(SC)]
            krot = [workp.tile([P, HD], BF16, tag=f"kr{s}", name=f"kr{s}") for s in range(SC)]
            vaug = [workp.tile([P, HD + 1], BF16, tag=f"va{s}", name=f"va{s}") for s in range(SC)]

            def rope(ps, dst, s, nh):
                # ps: PSUM [128, nh*64] f32; dst: SBUF bf16 same shape
                pv = ps[:].rearrange("p (h t c) -> p h t c", h=nh, t=2)
                dv = dst[:].rearrange("p (h t c) -> p h t c", h=nh, t=2)
                cs = cst[:, s * 256 : s * 256 + nh * 32].rearrange(
                    "p (h c) -> p h c", h=nh
                )
                sn = snt[:, s * 256 : s * 256 + nh * 32].rearrange(
                    "p (h c) -> p h c", h=nh
                )
                lo, hi = pv[:, :, 0, :], pv[:, :, 1, :]
                t1 = rtp.tile([P, NH * 32], F32, tag="rt1")
                t2 = rtp.tile([P, NH * 32], F32, tag="rt2")
                t1v = t1[:, : nh * 32].rearrange("p (h c) -> p h c", h=nh)
                t2v = t2[:, : nh * 32].rearrange("p (h c) -> p h c", h=nh)
                nc.any.tensor_mul(t1v, lo, cs)
                nc.any.tensor_mul(t2v, hi, sn)
                nc.any.tensor_sub(dv[:, :, 0, :], t1v, t2v)
                nc.any.tensor_mul(t1v, hi, cs)
                nc.any.tensor_mul(t2v, lo, sn)
                nc.any.tensor_add(dv[:, :, 1, :], t1v, t2v)

            for s in range(SC):
                pk = psump.tile([P, HD], F32, tag="mm512", bufs=2)
                for d in range(DC):
                    nc.tensor.matmul(
                        pk[:],
                        xT[d][:, s * P : (s + 1) * P],
                        wk_all[:, d * HD : (d + 1) * HD],
                        start=(d == 0),
                        stop=(d == DC - 1),
                    )
                rope(pk, krot[s], s, 1)

                pv_ = psump.tile([P, HD], F32, tag="mm512", bufs=2)
                for d in range(DC):
                    nc.tensor.matmul(
                        pv_[:],
                        xT[d][:, s * P : (s + 1) * P],
                        wv_all[:, d * HD : (d + 1) * HD],
                        start=(d == 0),
                        stop=(d == DC - 1),
                    )
                nc.any.tensor_copy(vaug[s][:, 0:HD], pv_[:])
                nc.any.memset(vaug[s][:, HD : HD + 1], 1.0)

                pq = psump.tile([P, NH * HD], F32, tag="mm512", bufs=2)
                for d in range(DC):
                    nc.tensor.matmul(
                        pq[:],
                        xT[d][:, s * P : (s + 1) * P],
                        wq_all[:, d * 512 : (d + 1) * 512],
                        start=(d == 0),
                        stop=(d == DC - 1),
                    )
                rope(pq, qrot[s], s, NH)

            # ---- per-head transposes into one combined qT tile + kT ----
            # qTall columns: h*1024 + c*512 + a*128 + i  (s-chunk = 4c + a)
            qTall = workp.tile([64, NH * S], BF16, tag="qTall")
            kT = workp.tile([64, S], BF16, tag="kT")
            for s in range(SC):
                tpk = psump.tile([64, P], BF16, tag="sc", bufs=2)
                nc.tensor.transpose(tpk[:], krot[s][:], ident[:])
                nc.any.tensor_copy(kT[:, s * P : (s + 1) * P], tpk[:])
                for h in range(NH):
                    tpq = psump.tile([64, P], BF16, tag="sc", bufs=2)
                    nc.tensor.transpose(
                        tpq[:], qrot[s][:, h * HD : (h + 1) * HD], ident[:]
                    )
                    nc.any.tensor_copy(
                        qTall[:, h * S + s * P : h * S + (s + 1) * P], tpq[:]
                    )

            # view: [64, a(4), c(2), h(8), i(128)]; a scores matmul's rhs takes
            # (c, h, i)-ordered columns: [j:h | j:h' | j+4:h | j+4:h']
            qview = qTall[:].rearrange("p (h c a b) -> p a c h b", h=NH, c=2, a=4, b=P)

            # ---- attention (2 heads per matmul) + per-pair o_proj and RS ----
            oT = [workp.tile([P, S], BF16, tag=f"oT{c}", name=f"oT{c}") for c in range(4)]
            rs_in = [dramp.tile([256, D], BF16, name=f"rsin{j}") for j in range(4)]
            rs_out = [dramp.tile([64, D], BF16, name=f"rsout{j}") for j in range(4)]

            for j in range(4):  # q-block pair (j, j+4)
                for hp in range(NH // 2):  # head pair (2hp, 2hp+1)
                    oa_a = psump.tile([HD + 1, 256], F32, tag="oa", bufs=4)
                    oa_b = psump.tile([HD + 1, 256], F32, tag="oa", bufs=4)
                    for skc in range(j + 5):
                        both = skc <= j
                        ncols = 512 if both else 256
                        sc_t = psump.tile([P, ncols], F32, tag="sc", bufs=2)
                        at_t = attnp.tile([P, ncols], BF16, tag="at")
                        if both:
                            nc.tensor.matmul(
                                sc_t[:],
                                kT[:, skc * P : (skc + 1) * P],
                                qview[:, j, :, 2 * hp : 2 * hp + 2, :],
                                start=True,
                                stop=True,
                            )
                        else:
                            nc.tensor.matmul(
                                sc_t[:],
                                kT[:, skc * P : (skc + 1) * P],
                                qview[:, j, 1, 2 * hp : 2 * hp + 2, :],
                                start=True,
                                stop=True,
                            )
                        if skc == j:
                            for hh in range(2):
                                nc.any.tensor_add(
                                    sc_t[:, hh * P : (hh + 1) * P],
                                    sc_t[:, hh * P : (hh + 1) * P],
                                    mkt[:, j * P : (j + 1) * P],
                                )
                        if skc == j + 4:
                            off = ncols - 256
                            for hh in range(2):
                                nc.any.tensor_add(
                                    sc_t[:, off + hh * P : off + (hh + 1) * P],
                                    sc_t[:, off + hh * P : off + (hh + 1) * P],
                                    mkt[:, (j + 4) * P : (j + 5) * P],
                                )
                        nc.scalar.activation(
                            at_t[:],
                            sc_t[:],
                            mybir.ActivationFunctionType.Exp,
                            scale=0.125,
                        )
                        if both:
                            nc.tensor.matmul(
                                oa_a[:],
                                vaug[skc][:],
                                at_t[:, 0:256],
                                start=(skc == 0),
                                stop=(skc == j),
                            )
                        nc.tensor.matmul(
                            oa_b[:],
                            vaug[skc][:],
                            at_t[:, (ncols - 256) : ncols],
                            start=(skc == 0),
                            stop=(skc == j + 4),
                        )
                    # normalize and scatter into oT (c-major layout for o_proj)
                    for oa, qb in ((oa_a, j), (oa_b, j + 4)):
                        rec = tinyp.tile([1, 256], F32, tag="rec")
                        nc.vector.reciprocal(rec[:], oa[HD : HD + 1, :])
                        recb = tinyp.tile([1, 256], BF16, tag="recb")
                        nc.any.tensor_copy(recb[:], rec[:])
                        rb = psump.tile([64, 256], F32, tag="sc", bufs=2)
                        nc.tensor.matmul(rb[:], ones64[:], recb[:], start=True, stop=True)
                        rb_sb = tinyp.tile([64, 256], F32, tag="rbs")
                        nc.any.tensor_copy(rb_sb[:], rb[:])
                        for hh in range(2):
                            nc.vector.tensor_mul(
                                oT[hp][hh * 64 : (hh + 1) * 64, qb * P : (qb + 1) * P],
                                oa[0:HD, hh * P : (hh + 1) * P],
                                rb_sb[:, hh * P : (hh + 1) * P],
                            )
                # o_proj for the two finished s-chunks (j and j+4)
                for si, s in enumerate((j, j + 4)):
                    op_t = opartp.tile([P, D], BF16, tag="op")
                    for n in range(4):
                        po = psump.tile([P, 512], F32, tag="mm512", bufs=2)
                        for c in range(4):
                            nc.tensor.matmul(
                                po[:],
                                oT[c][:, s * P : (s + 1) * P],
                                wo_all[:, c * D + n * 512 : c * D + (n + 1) * 512],
                                start=(c == 0),
                                stop=(c == 3),
                            )
                        nc.any.tensor_copy(op_t[:, n * 512 : (n + 1) * 512], po[:])
                    nc.sync.dma_start(
                        out=rs_in[j][si * P : (si + 1) * P, :], in_=op_t[:]
                    )
                nc.gpsimd.collective_compute(
                    "ReduceScatter",
                    mybir.AluOpType.add,
                    replica_groups=GROUPS,
                    ins=[rs_in[j].opt()],
                    outs=[rs_out[j].opt()],
                )
                # final cast bf16 -> f32, DRAM -> DRAM
                nc.gpsimd.dma_start(
                    out=out_p[j * 64 : (j + 1) * 64, :], in_=rs_out[j][:, :]
                )

    nc.compile()
    return nc


def _get_nc():
    if "nc" not in _NC_CACHE:
        _NC_CACHE["nc"] = _build_graph()
    return _NC_CACHE["nc"]


def _shard_inputs(x, wq, wk, wv, wo, cos, sin, mask, pos):
    x = np.asarray(x, dtype=np.float32)
    wq = np.asarray(wq, dtype=np.float32)
    wk = np.asarray(wk, dtype=np.float32)
    wv = np.asarray(wv, dtype=np.float32)
    wo = np.asarray(wo, dtype=np.float32)
    cos = np.asarray(cos, dtype=np.float32)
    sin = np.asarray(sin, dtype=np.float32)
    mask = np.asarray(mask, dtype=np.float32)
    p = int(pos)

    cs = cos[p : p + S]  # [S, 32]
    sn = sin[p : p + S]
    cs8 = np.ascontiguousarray(np.tile(cs, (1, NH)))  # [S, 256]
    sn8 = np.ascontiguousarray(np.tile(sn, (1, NH)))
    # transposed diagonal 128x128 blocks of the additive mask, pre-scaled by
    # sqrt(HD) so exp(scale*(scores + 8*mask)) == exp(scores/8 + mask)
    mk8 = np.stack(
        [8.0 * mask[j * P : (j + 1) * P, j * P : (j + 1) * P].T for j in range(SC)]
    ).astype(np.float32)
    mk8 = np.ascontiguousarray(mk8)

    in_maps = []
    for d in range(N_CORES):
        g, kv = d // 4, d % 4
        in_maps.append(
            {
                "x": np.ascontiguousarray(x[g]),
                "wq": np.ascontiguousarray(wq[:, kv * 512 : (kv + 1) * 512]),
                "wk": np.ascontiguousarray(wk[:, kv * HD : (kv + 1) * HD]),
                "wv": np.ascontiguousarray(wv[:, kv * HD : (kv + 1) * HD]),
                "wo": np.ascontiguousarray(wo[kv * 512 : (kv + 1) * 512, :]),
                "cs8": cs8,
                "sn8": sn8,
                "mk8": mk8,
            }
        )
    return in_maps


def _run(inputs, trace=False, trace_kwargs=None):
    nc = _get_nc()
    in_maps = _shard_inputs(**inputs)
    res = run_bass_kernel_spmd(
        nc,
        in_maps,
        core_ids=list(range(N_CORES)),
        trace=trace,
        **(trace_kwargs or {}),
    )
    B = 2
    out = np.empty((B, S, D), dtype=np.float32)
    for d in range(N_CORES):
        g, r = d // 4, d % 4
        core_out = res.results[d]["out"]  # [256, D]; rows 64j.. = pair-j shard r
        for j in range(4):
            blk = core_out[j * 64 : (j + 1) * 64, :]
            if r < 2:
                out[g, j * P + r * 64 : j * P + (r + 1) * 64, :] = blk
            else:
                out[g, (j + 4) * P + (r - 2) * 64 : (j + 4) * P + (r - 1) * 64, :] = blk
    return out, res


def kernel(**inputs) -> np.ndarray:
    out, _ = _run(inputs, trace=False)
    return out


# revision 4
# speedup vs baseline: 1.0796x; 1.0796x over previous
# Distributed GQA attention prefill kernel for one TRN2 chip (8 NeuronCores).
#
# Problem: B=2, S=1024, D=2048, H=32 q-heads, KV=4 kv-heads, HD=64, causal,
# RoPE, f32 I/O. Sharding: core d = (batch g=d//4, kv-head kv=d%4). Each core
# computes q-proj for its 8 q heads, k/v-proj for its kv head, attention, and
# a partial o_proj ([S, 512] @ wo[512kv:512kv+512, :]). Four bf16
# ReduceScatters (one per q-block pair, rows of s-chunks {j, j+4} stacked)
# over each group of 4 cores sum the partials while later pairs compute.
#
# Compute runs in bf16 (f32 PSUM accumulation); f32 matmul on TRN2 is 4
# cycles/row vs 1 for bf16. All DRAM loads are gpsimd casting DMAs
# (f32 -> bf16 on the fly). Attention processes 2 heads per matmul (they
# share the kv head) to halve instruction count and keep the PE pipelined.
import sys

import numpy as np

try:
    import concourse.bass as bass  # noqa: F401
except ImportError:
    for p in ("/opt/trn_rl_repo", "/root/.axon_site/_ro/trn_rl_repo"):
        if p not in sys.path:
            sys.path.append(p)
    import concourse.bass as bass  # noqa: F401

import concourse.bacc as bacc
import concourse.mybir as mybir
import concourse.tile as tile
from concourse import masks
from concourse.bass_utils import run_bass_kernel_spmd

S = 1024
D = 2048
H = 32
KV = 4
HD = 64
NH = 8  # q heads per core
P = 128
SC = S // P  # 8 seq chunks
DC = D // P  # 16 D chunks
N_CORES = 8
GROUPS = [[0, 1, 2, 3], [4, 5, 6, 7]]

F32 = mybir.dt.float32
BF16 = mybir.dt.bfloat16

_NC_CACHE = {}


def _build_graph():
    nc = bacc.Bacc("TRN2", target_bir_lowering=False, debug=False, num_devices=N_CORES)

    x_p = nc.dram_tensor("x", [S, D], F32, kind="ExternalInput")
    wq_p = nc.dram_tensor("wq", [D, NH * HD], F32, kind="ExternalInput")
    wk_p = nc.dram_tensor("wk", [D, HD], F32, kind="ExternalInput")
    wv_p = nc.dram_tensor("wv", [D, HD], F32, kind="ExternalInput")
    wo_p = nc.dram_tensor("wo", [NH * HD, D], F32, kind="ExternalInput")
    cs_p = nc.dram_tensor("cs8", [S, NH * 32], F32, kind="ExternalInput")
    sn_p = nc.dram_tensor("sn8", [S, NH * 32], F32, kind="ExternalInput")
    mk_p = nc.dram_tensor("mk8", [SC, P, P], F32, kind="ExternalInput")
    out_p = nc.dram_tensor("out", [S // 4, D], F32, kind="ExternalOutput")

    with tile.TileContext(nc) as tc:
        with (
            tc.tile_pool(name="const", bufs=1) as constp,
            tc.tile_pool(name="big", bufs=1) as bigp,
            tc.tile_pool(name="work", bufs=1) as workp,
            tc.tile_pool(name="rt", bufs=4) as rtp,
            tc.tile_pool(name="attn", bufs=3) as attnp,
            tc.tile_pool(name="opart", bufs=2) as opartp,
            tc.tile_pool(name="tiny", bufs=4) as tinyp,
            tc.tile_pool(name="psum", bufs=1, space="PSUM") as psump,
            tc.tile_pool(name="dram", bufs=1, space="DRAM") as dramp,
        ):
            # ---- constants (cheap engine work first; DMAs ordered by need) ----
            ident = constp.tile([P, P], BF16, tag="ident")
            masks.make_identity(nc, ident[:])
            ones64 = constp.tile([1, 64], BF16, tag="ones64")
            nc.any.memset(ones64[:], 1.0)

            # x loads split per s-chunk so transposes can start early
            xt_all = bigp.tile([P, SC * D], BF16, tag="x_all")  # x[128s+p, d] at (p, s, d)
            for s in range(SC):
                nc.gpsimd.dma_start(
                    out=xt_all[:, s * D : (s + 1) * D],
                    in_=x_p[s * P : (s + 1) * P, :],
                )
            wk_all = bigp.tile([P, DC * HD], BF16, tag="wk_all")
            nc.gpsimd.dma_start(
                out=wk_all[:].rearrange("p (c n) -> p c n", c=DC),
                in_=wk_p[:, :].rearrange("(c p) n -> p c n", p=P),
            )
            wv_all = bigp.tile([P, DC * HD], BF16, tag="wv_all")
            nc.gpsimd.dma_start(
                out=wv_all[:].rearrange("p (c n) -> p c n", c=DC),
                in_=wv_p[:, :].rearrange("(c p) n -> p c n", p=P),
            )
            wq_all = bigp.tile([P, DC * 512], BF16, tag="wq_all")
            nc.gpsimd.dma_start(
                out=wq_all[:].rearrange("p (c n) -> p c n", c=DC),
                in_=wq_p[:, :].rearrange("(c p) n -> p c n", p=P),
            )
            cst = constp.tile([P, SC * 256], F32, tag="cst")  # cos, tiled x8 heads
            nc.gpsimd.dma_start(
                out=cst[:].rearrange("p (s c) -> p s c", s=SC),
                in_=cs_p[:, :].rearrange("(s p) c -> p s c", p=P),
            )
            snt = constp.tile([P, SC * 256], F32, tag="snt")
            nc.gpsimd.dma_start(
                out=snt[:].rearrange("p (s c) -> p s c", s=SC),
                in_=sn_p[:, :].rearrange("(s p) c -> p s c", p=P),
            )
            mkt = constp.tile([P, SC * P], F32, tag="mkt")  # 8 transposed diag blocks *8
            nc.gpsimd.dma_start(
                out=mkt[:].rearrange("p (j c) -> p j c", j=SC),
                in_=mk_p[:, :, :].rearrange("j p c -> p j c"),
            )
            wo_all = bigp.tile([P, 4 * D], BF16, tag="wo_all")
            nc.gpsimd.dma_start(
                out=wo_all[:].rearrange("p (c n) -> p c n", c=4),
                in_=wo_p[:, :].rearrange("(c p) n -> p c n", p=P),
            )

            # ---- xT: transpose x to [D, S] layout (bf16, via PE) ----
            xT = [workp.tile([P, S], BF16, tag=f"xT{d}", name=f"xT{d}") for d in range(DC)]
            for s in range(SC):
                for d in range(DC):
                    tp = psump.tile([P, P], BF16, tag="sc", bufs=2)
                    nc.tensor.transpose(
                        tp[:], xt_all[:, s * D + d * P : s * D + (d + 1) * P], ident[:]
                    )
                    nc.vector.tensor_copy(xT[d][:, s * P : (s + 1) * P], tp[:])

            # ---- projections + RoPE (natural [s, ch] layout) ----
            qrot = [workp.tile([P, NH * HD], BF16, tag=f"qr{s}", name=f"qr{s}") for s in range(SC)]
            krot = [workp.tile([P, HD], BF16, tag=f"kr{s}", name=f"kr{s}") for s in range(SC)]
            vaug = [workp.tile([P, HD + 1], BF16, tag=f"va{s}", name=f"va{s}") for s in range(SC)]

            def rope(ps, dst, s, nh):
                # ps: PSUM [128, nh*64] f32; dst: SBUF bf16 same shape
                pv = ps[:].rearrange("p (h t c) -> p h t c", h=nh, t=2)
                dv = dst[:].rearrange("p (h t c) -> p h t c", h=nh, t=2)
                cs = cst[:, s * 256 : s * 256 + nh * 32].rearrange(
                    "p (h c) -> p h c", h=nh
                )
                sn = snt[:, s * 256 : s * 256 + nh * 32].rearrange(
                    "p (h c) -> p h c", h=nh
                )
                lo, hi = pv[:, :, 0, :], pv[:, :, 1, :]
                t1 = rtp.tile([P, NH * 32], F32, tag="rt1")
                t2 = rtp.tile([P, NH * 32], F32, tag="rt2")
                t1v = t1[:, : nh * 32].rearrange("p (h c) -> p h c", h=nh)
                t2v = t2[:, : nh * 32].rearrange("p (h c) -> p h c", h=nh)
                nc.any.tensor_mul(t1v, lo, cs)
                nc.any.tensor_mul(t2v, hi, sn)
                nc.any.tensor_sub(dv[:, :, 0, :], t1v, t2v)
                nc.any.tensor_mul(t1v, hi, cs)
                nc.any.tensor_mul(t2v, lo, sn)
                nc.any.tensor_add(dv[:, :, 1, :], t1v, t2v)

            for s in range(SC):
                pk = psump.tile([P, HD], F32, tag="mm512", bufs=2)
                for d in range(DC):
                    nc.tensor.matmul(
                        pk[:],
                        xT[d][:, s * P : (s + 1) * P],
                        wk_all[:, d * HD : (d + 1) * HD],
                        start=(d == 0),
                        stop=(d == DC - 1),
                    )
                rope(pk, krot[s], s, 1)

                pv_ = psump.tile([P, HD], F32, tag="mm512", bufs=2)
                for d in range(DC):
                    nc.tensor.matmul(
                        pv_[:],
                        xT[d][:, s * P : (s + 1) * P],
                        wv_all[:, d * HD : (d + 1) * HD],
                        start=(d == 0),
                        stop=(d == DC - 1),
                    )
                nc.vector.tensor_copy(vaug[s][:, 0:HD], pv_[:])
                nc.any.memset(vaug[s][:, HD : HD + 1], 1.0)

                pq = psump.tile([P, NH * HD], F32, tag="mm512", bufs=2)
                for d in range(DC):
                    nc.tensor.matmul(
                        pq[:],
                        xT[d][:, s * P : (s + 1) * P],
                        wq_all[:, d * 512 : (d + 1) * 512],
                        start=(d == 0),
                        stop=(d == DC - 1),
                    )
                rope(pq, qrot[s], s, NH)

            # ---- per-head transposes into one combined qT tile + kT ----
            # qTall columns: h*1024 + c*512 + a*128 + i  (s-chunk = 4c + a)
            qTall = workp.tile([64, NH * S], BF16, tag="qTall")
            kT = workp.tile([64, S], BF16, tag="kT")
            for s in range(SC):
                tpk = psump.tile([64, P], BF16, tag="sc", bufs=2)
                nc.tensor.transpose(tpk[:], krot[s][:], ident[:])
                nc.vector.tensor_copy(kT[:, s * P : (s + 1) * P], tpk[:])
                for h in range(NH):
                    tpq = psump.tile([64, P], BF16, tag="sc", bufs=2)
                    nc.tensor.transpose(
                        tpq[:], qrot[s][:, h * HD : (h + 1) * HD], ident[:]
                    )
                    nc.vector.tensor_copy(
                        qTall[:, h * S + s * P : h * S + (s + 1) * P], tpq[:]
                    )

            # view: [64, a(4), c(2), h(8), i(128)]; a scores matmul's rhs takes
            # (c, h, i)-ordered columns: [j:h | j:h' | j+4:h | j+4:h']
            qview = qTall[:].rearrange("p (h c a b) -> p a c h b", h=NH, c=2, a=4, b=P)

            # ---- attention (2 heads per matmul) + per-pair o_proj and RS ----
            oT = [workp.tile([P, S], BF16, tag=f"oT{c}", name=f"oT{c}") for c in range(4)]
            rs_in = [dramp.tile([256, D], BF16, name=f"rsin{j}") for j in range(4)]
            rs_out = [dramp.tile([64, D], BF16, name=f"rsout{j}") for j in range(4)]

            for j in range(4):  # q-block pair (j, j+4)
                for hp in range(NH // 2):  # head pair (2hp, 2hp+1)
                    # phase 1: stream all score matmuls + exp into SBUF
                    ats = []
                    for skc in range(j + 5):
                        both = skc <= j
                        ncols = 512 if both else 256
                        sc_t = psump.tile([P, ncols], F32, tag="sc", bufs=2)
                        at_t = attnp.tile([P, ncols], BF16, tag="at", bufs=9)
                        if both:
                            nc.tensor.matmul(
                                sc_t[:],
                                kT[:, skc * P : (skc + 1) * P],
                                qview[:, j, :, 2 * hp : 2 * hp + 2, :],
                                start=True,
                                stop=True,
                            )
                        else:
                            nc.tensor.matmul(
                                sc_t[:],
                                kT[:, skc * P : (skc + 1) * P],
                                qview[:, j, 1, 2 * hp : 2 * hp + 2, :],
                                start=True,
                                stop=True,
                            )
                        if skc == j:
                            for hh in range(2):
                                nc.vector.tensor_add(
                                    sc_t[:, hh * P : (hh + 1) * P],
                                    sc_t[:, hh * P : (hh + 1) * P],
                                    mkt[:, j * P : (j + 1) * P],
                                )
                        if skc == j + 4:
                            off = ncols - 256
                            for hh in range(2):
                                nc.vector.tensor_add(
                                    sc_t[:, off + hh * P : off + (hh + 1) * P],
                                    sc_t[:, off + hh * P : off + (hh + 1) * P],
                                    mkt[:, (j + 4) * P : (j + 5) * P],
                                )
                        nc.scalar.activation(
                            at_t[:],
                            sc_t[:],
                            mybir.ActivationFunctionType.Exp,
                            scale=0.125,
                        )
                        ats.append((at_t, ncols))
                    # phase 2: dense attnv accumulation runs
                    oa_a = psump.tile([HD + 1, 256], F32, tag="oa", bufs=4)
                    oa_b = psump.tile([HD + 1, 256], F32, tag="oa", bufs=4)
                    for skc in range(j + 5):
                        at_t, ncols = ats[skc]
                        if skc <= j:
                            nc.tensor.matmul(
                                oa_a[:],
                                vaug[skc][:],
                                at_t[:, 0:256],
                                start=(skc == 0),
                                stop=(skc == j),
                            )
                    for skc in range(j + 5):
                        at_t, ncols = ats[skc]
                        nc.tensor.matmul(
                            oa_b[:],
                            vaug[skc][:],
                            at_t[:, (ncols - 256) : ncols],
                            start=(skc == 0),
                            stop=(skc == j + 4),
                        )
                    # normalize and scatter into oT (c-major layout for o_proj)
                    rec = tinyp.tile([1, 512], F32, tag="rec")
                    nc.vector.reciprocal(rec[:, 0:256], oa_a[HD : HD + 1, :])
                    nc.vector.reciprocal(rec[:, 256:512], oa_b[HD : HD + 1, :])
                    recb = tinyp.tile([1, 512], BF16, tag="recb")
                    nc.vector.tensor_copy(recb[:], rec[:])
                    rb = psump.tile([64, 512], F32, tag="sc", bufs=2)
                    nc.tensor.matmul(rb[:], ones64[:], recb[:], start=True, stop=True)
                    rb_sb = tinyp.tile([64, 512], F32, tag="rbs")
                    nc.vector.tensor_copy(rb_sb[:], rb[:])
                    for oi, (oa, qb) in enumerate(((oa_a, j), (oa_b, j + 4))):
                        for hh in range(2):
                            nc.vector.tensor_mul(
                                oT[hp][hh * 64 : (hh + 1) * 64, qb * P : (qb + 1) * P],
                                oa[0:HD, hh * P : (hh + 1) * P],
                                rb_sb[:, oi * 256 + hh * P : oi * 256 + (hh + 1) * P],
                            )
                # o_proj for the two finished s-chunks (j and j+4)
                for si, s in enumerate((j, j + 4)):
                    op_t = opartp.tile([P, D], BF16, tag="op")
                    for n in range(4):
                        po = psump.tile([P, 512], F32, tag="mm512", bufs=2)
                        for c in range(4):
                            nc.tensor.matmul(
                                po[:],
                                oT[c][:, s * P : (s + 1) * P],
                                wo_all[:, c * D + n * 512 : c * D + (n + 1) * 512],
                                start=(c == 0),
                                stop=(c == 3),
                            )
                        nc.vector.tensor_copy(op_t[:, n * 512 : (n + 1) * 512], po[:])
                    nc.sync.dma_start(
                        out=rs_in[j][si * P : (si + 1) * P, :], in_=op_t[:]
                    )
                nc.gpsimd.collective_compute(
                    "ReduceScatter",
                    mybir.AluOpType.add,
                    replica_groups=GROUPS,
                    ins=[rs_in[j].opt()],
                    outs=[rs_out[j].opt()],
                )
                # final cast bf16 -> f32, DRAM -> DRAM
                nc.gpsimd.dma_start(
                    out=out_p[j * 64 : (j + 1) * 64, :], in_=rs_out[j][:, :]
                )

    nc.compile()
    return nc


def _get_nc():
    if "nc" not in _NC_CACHE:
        _NC_CACHE["nc"] = _build_graph()
    return _NC_CACHE["nc"]


def _shard_inputs(x, wq, wk, wv, wo, cos, sin, mask, pos):
    x = np.asarray(x, dtype=np.float32)
    wq = np.asarray(wq, dtype=np.float32)
    wk = np.asarray(wk, dtype=np.float32)
    wv = np.asarray(wv, dtype=np.float32)
    wo = np.asarray(wo, dtype=np.float32)
    cos = np.asarray(cos, dtype=np.float32)
    sin = np.asarray(sin, dtype=np.float32)
    mask = np.asarray(mask, dtype=np.float32)
    p = int(pos)

    cs = cos[p : p + S]  # [S, 32]
    sn = sin[p : p + S]
    cs8 = np.ascontiguousarray(np.tile(cs, (1, NH)))  # [S, 256]
    sn8 = np.ascontiguousarray(np.tile(sn, (1, NH)))
    # transposed diagonal 128x128 blocks of the additive mask, pre-scaled by
    # sqrt(HD) so exp(scale*(scores + 8*mask)) == exp(scores/8 + mask)
    mk8 = np.stack(
        [8.0 * mask[j * P : (j + 1) * P, j * P : (j + 1) * P].T for j in range(SC)]
    ).astype(np.float32)
    mk8 = np.ascontiguousarray(mk8)

    in_maps = []
    for d in range(N_CORES):
        g, kv = d // 4, d % 4
        in_maps.append(
            {
                "x": np.ascontiguousarray(x[g]),
                "wq": np.ascontiguousarray(wq[:, kv * 512 : (kv + 1) * 512]),
                "wk": np.ascontiguousarray(wk[:, kv * HD : (kv + 1) * HD]),
                "wv": np.ascontiguousarray(wv[:, kv * HD : (kv + 1) * HD]),
                "wo": np.ascontiguousarray(wo[kv * 512 : (kv + 1) * 512, :]),
                "cs8": cs8,
                "sn8": sn8,
                "mk8": mk8,
            }
        )
    return in_maps


def _run(inputs, trace=False, trace_kwargs=None):
    nc = _get_nc()
    in_maps = _shard_inputs(**inputs)
    res = run_bass_kernel_spmd(
        nc,
        in_maps,
        core_ids=list(range(N_CORES)),
        trace=trace,
        **(trace_kwargs or {}),
    )
    B = 2
    out = np.empty((B, S, D), dtype=np.float32)
    for d in range(N_CORES):
        g, r = d // 4, d % 4
        core_out = res.results[d]["out"]  # [256, D]; rows 64j.. = pair-j shard r
        for j in range(4):
            blk = core_out[j * 64 : (j + 1) * 64, :]
            if r < 2:
                out[g, j * P + r * 64 : j * P + (r + 1) * 64, :] = blk
            else:
                out[g, (j + 4) * P + (r - 2) * 64 : (j + 4) * P + (r - 1) * 64, :] = blk
    return out, res


def kernel(**inputs) -> np.ndarray:
    out, _ = _run(inputs, trace=False)
    return out


# revision 7
# speedup vs baseline: 1.0848x; 1.0048x over previous
# Distributed GQA attention prefill kernel for one TRN2 chip (8 NeuronCores).
#
# Problem: B=2, S=1024, D=2048, H=32 q-heads, KV=4 kv-heads, HD=64, causal,
# RoPE, f32 I/O. Sharding: core d = (batch g=d//4, kv-head kv=d%4). Each core
# computes q-proj for its 8 q heads, k/v-proj for its kv head, attention, and
# a partial o_proj ([S, 512] @ wo[512kv:512kv+512, :]). Four bf16
# ReduceScatters (one per q-block pair, rows of s-chunks {j, j+4} stacked)
# over each group of 4 cores sum the partials while later pairs compute.
#
# Compute runs in bf16 (f32 PSUM accumulation); f32 matmul on TRN2 is 4
# cycles/row vs 1 for bf16. All DRAM loads are gpsimd casting DMAs
# (f32 -> bf16 on the fly). Attention processes 2 heads per matmul (they
# share the kv head) to halve instruction count and keep the PE pipelined.
import sys

import numpy as np

try:
    import concourse.bass as bass  # noqa: F401
except ImportError:
    for p in ("/opt/trn_rl_repo", "/root/.axon_site/_ro/trn_rl_repo"):
        if p not in sys.path:
            sys.path.append(p)
    import concourse.bass as bass  # noqa: F401

import concourse.bacc as bacc
import concourse.mybir as mybir
import concourse.tile as tile
from concourse import masks
from concourse.bass_utils import run_bass_kernel_spmd

S = 1024
D = 2048
H = 32
KV = 4
HD = 64
NH = 8  # q heads per core
P = 128
SC = S // P  # 8 seq chunks
DC = D // P  # 16 D chunks
N_CORES = 8
GROUPS = [[0, 1, 2, 3], [4, 5, 6, 7]]

F32 = mybir.dt.float32
BF16 = mybir.dt.bfloat16

_NC_CACHE = {}


def _build_graph():
    nc = bacc.Bacc("TRN2", target_bir_lowering=False, debug=False, num_devices=N_CORES)

    x_p = nc.dram_tensor("x", [S, D], F32, kind="ExternalInput")
    wq_p = nc.dram_tensor("wq", [D, NH * HD], F32, kind="ExternalInput")
    wkv_p = nc.dram_tensor("wkv", [D, 2 * HD], F32, kind="ExternalInput")
    wo_p = nc.dram_tensor("wo", [NH * HD, D], F32, kind="ExternalInput")
    cs_p = nc.dram_tensor("cs8", [S, NH * 32], F32, kind="ExternalInput")
    sn_p = nc.dram_tensor("sn8", [S, NH * 32], F32, kind="ExternalInput")
    mk_p = nc.dram_tensor("mk8", [SC, P, P], F32, kind="ExternalInput")
    out_p = nc.dram_tensor("out", [S // 4, D], F32, kind="ExternalOutput")

    with tile.TileContext(nc) as tc:
        with (
            tc.tile_pool(name="const", bufs=1) as constp,
            tc.tile_pool(name="big", bufs=1) as bigp,
            tc.tile_pool(name="work", bufs=1) as workp,
            tc.tile_pool(name="rt", bufs=4) as rtp,
            tc.tile_pool(name="attn", bufs=3) as attnp,
            tc.tile_pool(name="opart", bufs=2) as opartp,
            tc.tile_pool(name="tiny", bufs=4) as tinyp,
            tc.tile_pool(name="psum", bufs=1, space="PSUM") as psump,
            tc.tile_pool(name="dram", bufs=1, space="DRAM") as dramp,
        ):
            # ---- constants (cheap engine work first; DMAs ordered by need) ----
            ident = constp.tile([P, P], BF16, tag="ident")
            masks.make_identity(nc, ident[:])
            ones64 = constp.tile([1, 64], BF16, tag="ones64")
            nc.any.memset(ones64[:], 1.0)

            # x loads split per s-chunk so transposes can start early
            xt_all = bigp.tile([P, SC * D], BF16, tag="x_all")  # x[128s+p, d] at (p, s, d)

            def load_x(s):
                nc.gpsimd.dma_start(
                    out=xt_all[:, s * D : (s + 1) * D],
                    in_=x_p[s * P : (s + 1) * P, :],
                )

            load_x(0)
            wkv_all = bigp.tile([P, DC * 2 * HD], BF16, tag="wkv_all")
            nc.gpsimd.dma_start(
                out=wkv_all[:].rearrange("p (c n) -> p c n", c=DC),
                in_=wkv_p[:, :].rearrange("(c p) n -> p c n", p=P),
            )
            load_x(1)
            wq_all = bigp.tile([P, DC * 512], BF16, tag="wq_all")
            nc.gpsimd.dma_start(
                out=wq_all[:].rearrange("p (c n) -> p c n", c=DC),
                in_=wq_p[:, :].rearrange("(c p) n -> p c n", p=P),
            )
            for s in range(2, SC):
                load_x(s)
            cst = constp.tile([P, SC * 256], F32, tag="cst")  # cos, tiled x8 heads
            nc.gpsimd.dma_start(
                out=cst[:].rearrange("p (s c) -> p s c", s=SC),
                in_=cs_p[:, :].rearrange("(s p) c -> p s c", p=P),
            )
            snt = constp.tile([P, SC * 256], F32, tag="snt")
            nc.gpsimd.dma_start(
                out=snt[:].rearrange("p (s c) -> p s c", s=SC),
                in_=sn_p[:, :].rearrange("(s p) c -> p s c", p=P),
            )
            mkt = constp.tile([P, SC * P], F32, tag="mkt")  # 8 transposed diag blocks *8
            nc.gpsimd.dma_start(
                out=mkt[:].rearrange("p (j c) -> p j c", j=SC),
                in_=mk_p[:, :, :].rearrange("j p c -> p j c"),
            )
            wo_all = bigp.tile([P, 4 * D], BF16, tag="wo_all")
            nc.gpsimd.dma_start(
                out=wo_all[:].rearrange("p (c n) -> p c n", c=4),
                in_=wo_p[:, :].rearrange("(c p) n -> p c n", p=P),
            )

            # ---- xT: transpose x to [D, S] layout (bf16, via PE) ----
            xT = [workp.tile([P, S], BF16, tag=f"xT{d}", name=f"xT{d}") for d in range(DC)]
            for s in range(SC):
                for d in range(DC):
                    tp = psump.tile([P, P], BF16, tag="sc", bufs=3)
                    nc.tensor.transpose(
                        tp[:], xt_all[:, s * D + d * P : s * D + (d + 1) * P], ident[:]
                    )
                    nc.vector.tensor_copy(xT[d][:, s * P : (s + 1) * P], tp[:])

            # ---- projections + RoPE (natural [s, ch] layout) ----
            qrot = [workp.tile([P, NH * HD], BF16, tag=f"qr{s}", name=f"qr{s}") for s in range(SC)]
            krot = [workp.tile([P, HD], BF16, tag=f"kr{s}", name=f"kr{s}") for s in range(SC)]
            vaug = [workp.tile([P, HD + 1], BF16, tag=f"va{s}", name=f"va{s}") for s in range(SC)]

            def rope(ps_ap, dst, s, nh):
                # ps_ap: PSUM AP [128, nh*64] f32; dst: SBUF bf16 same shape
                pv = ps_ap.rearrange("p (h t c) -> p h t c", h=nh, t=2)
                dv = dst[:].rearrange("p (h t c) -> p h t c", h=nh, t=2)
                cs = cst[:, s * 256 : s * 256 + nh * 32].rearrange(
                    "p (h c) -> p h c", h=nh
                )
                sn = snt[:, s * 256 : s * 256 + nh * 32].rearrange(
                    "p (h c) -> p h c", h=nh
                )
                lo, hi = pv[:, :, 0, :], pv[:, :, 1, :]
                t1 = rtp.tile([P, NH * 32], F32, tag="rt1")
                t2 = rtp.tile([P, NH * 32], F32, tag="rt2")
                t1v = t1[:, : nh * 32].rearrange("p (h c) -> p h c", h=nh)
                t2v = t2[:, : nh * 32].rearrange("p (h c) -> p h c", h=nh)
                nc.any.tensor_mul(t1v, lo, cs)
                nc.any.tensor_mul(t2v, hi, sn)
                nc.any.tensor_sub(dv[:, :, 0, :], t1v, t2v)
                nc.any.tensor_mul(t1v, hi, cs)
                nc.any.tensor_mul(t2v, lo, sn)
                nc.any.tensor_add(dv[:, :, 1, :], t1v, t2v)

            for s in range(SC):
                pkv = psump.tile([P, 2 * HD], F32, tag="mm512", bufs=3)
                for d in range(DC):
                    nc.tensor.matmul(
                        pkv[:],
                        xT[d][:, s * P : (s + 1) * P],
                        wkv_all[:, d * 2 * HD : (d + 1) * 2 * HD],
                        start=(d == 0),
                        stop=(d == DC - 1),
                    )
                rope(pkv[:, 0:HD], krot[s], s, 1)
                nc.vector.tensor_copy(vaug[s][:, 0:HD], pkv[:, HD : 2 * HD])
                nc.any.memset(vaug[s][:, HD : HD + 1], 1.0)

                pq = psump.tile([P, NH * HD], F32, tag="mm512", bufs=3)
                for d in range(DC):
                    nc.tensor.matmul(
                        pq[:],
                        xT[d][:, s * P : (s + 1) * P],
                        wq_all[:, d * 512 : (d + 1) * 512],
                        start=(d == 0),
                        stop=(d == DC - 1),
                    )
                rope(pq[:, :], qrot[s], s, NH)

            # ---- per-head transposes into one combined qT tile + kT ----
            # qTall columns: h*1024 + c*512 + a*128 + i  (s-chunk = 4c + a)
            qTall = workp.tile([64, NH * S], BF16, tag="qTall")
            kT = workp.tile([64, S], BF16, tag="kT")
            for s in range(SC):
                tpk = psump.tile([64, P], BF16, tag="sc", bufs=3)
                nc.tensor.transpose(tpk[:], krot[s][:], ident[:])
                nc.vector.tensor_copy(kT[:, s * P : (s + 1) * P], tpk[:])
                for h in range(NH):
                    tpq = psump.tile([64, P], BF16, tag="sc", bufs=3)
                    nc.tensor.transpose(
                        tpq[:], qrot[s][:, h * HD : (h + 1) * HD], ident[:]
                    )
                    nc.vector.tensor_copy(
                        qTall[:, h * S + s * P : h * S + (s + 1) * P], tpq[:]
                    )

            # view: [64, a(4), c(2), h(8), i(128)]; a scores matmul's rhs takes
            # (c, h, i)-ordered columns: [j:h | j:h' | j+4:h | j+4:h']
            qview = qTall[:].rearrange("p (h c a b) -> p a c h b", h=NH, c=2, a=4, b=P)

            def _normalize(j, oa, hp):
                # oa: PSUM [65, 512]; row 64 = softmax denominators
                rec = tinyp.tile([1, 512], F32, tag="rec")
                nc.vector.reciprocal(rec[:], oa[HD : HD + 1, :])
                recb = tinyp.tile([1, 512], BF16, tag="recb")
                nc.vector.tensor_copy(recb[:], rec[:])
                rb = psump.tile([64, 512], F32, tag="sc", bufs=3)
                nc.tensor.matmul(rb[:], ones64[:], recb[:], start=True, stop=True)
                rb_sb = tinyp.tile([64, 512], F32, tag="rbs")
                nc.vector.tensor_copy(rb_sb[:], rb[:])
                for oi, qb in ((0, j), (1, j + 4)):
                    for hh in range(2):
                        nc.vector.tensor_mul(
                            oT[hp][hh * 64 : (hh + 1) * 64, qb * P : (qb + 1) * P],
                            oa[0:HD, oi * 256 + hh * P : oi * 256 + (hh + 1) * P],
                            rb_sb[:, oi * 256 + hh * P : oi * 256 + (hh + 1) * P],
                        )

            # ---- attention (2 heads per matmul) + per-pair o_proj and RS ----
            oT = [workp.tile([P, S], BF16, tag=f"oT{c}", name=f"oT{c}") for c in range(4)]
            rs_in = [dramp.tile([256, D], BF16, name=f"rsin{j}") for j in range(4)]
            rs_out = [dramp.tile([64, D], BF16, name=f"rsout{j}") for j in range(4)]

            for j in range(4):  # q-block pair (j, j+4)
                pending = None  # deferred normalization: (oa, hp)
                for hp in range(NH // 2):  # head pair (2hp, 2hp+1)
                    # phase 1: stream all score matmuls + exp into SBUF
                    ats = []
                    for skc in range(j + 5):
                        both = skc <= j
                        ncols = 512 if both else 256
                        sc_t = psump.tile([P, ncols], F32, tag="sc", bufs=3)
                        at_t = attnp.tile([P, ncols], BF16, tag="at", bufs=9)
                        if both:
                            nc.tensor.matmul(
                                sc_t[:],
                                kT[:, skc * P : (skc + 1) * P],
                                qview[:, j, :, 2 * hp : 2 * hp + 2, :],
                                start=True,
                                stop=True,
                            )
                        else:
                            nc.tensor.matmul(
                                sc_t[:],
                                kT[:, skc * P : (skc + 1) * P],
                                qview[:, j, 1, 2 * hp : 2 * hp + 2, :],
                                start=True,
                                stop=True,
                            )
                        if skc == j:
                            for hh in range(2):
                                nc.vector.tensor_add(
                                    sc_t[:, hh * P : (hh + 1) * P],
                                    sc_t[:, hh * P : (hh + 1) * P],
                                    mkt[:, j * P : (j + 1) * P],
                                )
                        if skc == j + 4:
                            off = ncols - 256
                            for hh in range(2):
                                nc.vector.tensor_add(
                                    sc_t[:, off + hh * P : off + (hh + 1) * P],
                                    sc_t[:, off + hh * P : off + (hh + 1) * P],
                                    mkt[:, (j + 4) * P : (j + 5) * P],
                                )
                        nc.scalar.activation(
                            at_t[:],
                            sc_t[:],
                            mybir.ActivationFunctionType.Exp,
                            scale=0.125,
                        )
                        ats.append((at_t, ncols))
                    # phase 2: dense attnv accumulation runs into one PSUM bank
                    # (cols 0:256 = q-block j, 256:512 = q-block j+4)
                    oa = psump.tile([HD + 1, 512], F32, tag="oa", bufs=2)
                    for skc in range(j + 1):
                        at_t, ncols = ats[skc]
                        nc.tensor.matmul(
                            oa[:, 0:256],
                            vaug[skc][:],
                            at_t[:, 0:256],
                            start=(skc == 0),
                            stop=(skc == j),
                            skip_group_check=True,
                        )
                    for skc in range(j + 5):
                        at_t, ncols = ats[skc]
                        nc.tensor.matmul(
                            oa[:, 256:512],
                            vaug[skc][:],
                            at_t[:, (ncols - 256) : ncols],
                            start=(skc == 0),
                            stop=(skc == j + 4),
                            skip_group_check=True,
                        )
                    if pending is not None:
                        _normalize(j, *pending)
                    pending = (oa, hp)
                if pending is not None:
                    _normalize(j, *pending)
                # o_proj for the two finished s-chunks (j and j+4)
                for si, s in enumerate((j, j + 4)):
                    op_t = opartp.tile([P, D], BF16, tag="op")
                    for n in range(4):
                        po = psump.tile([P, 512], F32, tag="mm512", bufs=3)
                        for c in range(4):
                            nc.tensor.matmul(
                                po[:],
                                oT[c][:, s * P : (s + 1) * P],
                                wo_all[:, c * D + n * 512 : c * D + (n + 1) * 512],
                                start=(c == 0),
                                stop=(c == 3),
                            )
                        nc.scalar.copy(op_t[:, n * 512 : (n + 1) * 512], po[:])
                    nc.sync.dma_start(
                        out=rs_in[j][si * P : (si + 1) * P, :], in_=op_t[:]
                    )
                nc.gpsimd.collective_compute(
                    "ReduceScatter",
                    mybir.AluOpType.add,
                    replica_groups=GROUPS,
                    ins=[rs_in[j].opt()],
                    outs=[rs_out[j].opt()],
                )
                # final cast bf16 -> f32, DRAM -> DRAM
                nc.gpsimd.dma_start(
                    out=out_p[j * 64 : (j + 1) * 64, :], in_=rs_out[j][:, :]
                )

    nc.compile()
    return nc


def _get_nc():
    if "nc" not in _NC_CACHE:
        _NC_CACHE["nc"] = _build_graph()
    return _NC_CACHE["nc"]


def _shard_inputs(x, wq, wk, wv, wo, cos, sin, mask, pos):
    x = np.asarray(x, dtype=np.float32)
    wq = np.asarray(wq, dtype=np.float32)
    wk = np.asarray(wk, dtype=np.float32)
    wv = np.asarray(wv, dtype=np.float32)
    wo = np.asarray(wo, dtype=np.float32)
    cos = np.asarray(cos, dtype=np.float32)
    sin = np.asarray(sin, dtype=np.float32)
    mask = np.asarray(mask, dtype=np.float32)
    p = int(pos)

    cs = cos[p : p + S]  # [S, 32]
    sn = sin[p : p + S]
    cs8 = np.ascontiguousarray(np.tile(cs, (1, NH)))  # [S, 256]
    sn8 = np.ascontiguousarray(np.tile(sn, (1, NH)))
    # transposed diagonal 128x128 blocks of the additive mask, pre-scaled by
    # sqrt(HD) so exp(scale*(scores + 8*mask)) == exp(scores/8 + mask)
    mk8 = np.stack(
        [8.0 * mask[j * P : (j + 1) * P, j * P : (j + 1) * P].T for j in range(SC)]
    ).astype(np.float32)
    mk8 = np.ascontiguousarray(mk8)

    in_maps = []
    for d in range(N_CORES):
        g, kv = d // 4, d % 4
        in_maps.append(
            {
                "x": np.ascontiguousarray(x[g]),
                "wq": np.ascontiguousarray(wq[:, kv * 512 : (kv + 1) * 512]),
                "wkv": np.ascontiguousarray(
                    np.concatenate(
                        [
                            wk[:, kv * HD : (kv + 1) * HD],
                            wv[:, kv * HD : (kv + 1) * HD],
                        ],
                        axis=1,
                    )
                ),
                "wo": np.ascontiguousarray(wo[kv * 512 : (kv + 1) * 512, :]),
                "cs8": cs8,
                "sn8": sn8,
                "mk8": mk8,
            }
        )
    return in_maps


def _run(inputs, trace=False, trace_kwargs=None):
    nc = _get_nc()
    in_maps = _shard_inputs(**inputs)
    res = run_bass_kernel_spmd(
        nc,
        in_maps,
        core_ids=list(range(N_CORES)),
        trace=trace,
        **(trace_kwargs or {}),
    )
    B = 2
    out = np.empty((B, S, D), dtype=np.float32)
    for d in range(N_CORES):
        g, r = d // 4, d % 4
        core_out = res.results[d]["out"]  # [256, D]; rows 64j.. = pair-j shard r
        for j in range(4):
            blk = core_out[j * 64 : (j + 1) * 64, :]
            if r < 2:
                out[g, j * P + r * 64 : j * P + (r + 1) * 64, :] = blk
            else:
                out[g, (j + 4) * P + (r - 2) * 64 : (j + 4) * P + (r - 1) * 64, :] = blk
    return out, res


def kernel(**inputs) -> np.ndarray:
    out, _ = _run(inputs, trace=False)
    return out


# revision 8
# speedup vs baseline: 1.2127x; 1.1179x over previous
# Distributed GQA attention prefill kernel for one TRN2 chip (8 NeuronCores).
#
# Problem: B=2, S=1024, D=2048, H=32 q-heads, KV=4 kv-heads, HD=64, causal,
# RoPE, f32 I/O. Sharding: core d = (batch g=d//4, kv-head kv=d%4). Each core
# computes q-proj for its 8 q heads, k/v-proj for its kv head, attention, and
# a partial o_proj ([S, 512] @ wo[512kv:512kv+512, :]). Four bf16
# ReduceScatters (one per q-block pair, rows of s-chunks {j, j+4} stacked)
# over each group of 4 cores sum the partials while later pairs compute.
#
# Compute runs in bf16 (f32 PSUM accumulation); f32 matmul on TRN2 is 4
# cycles/row vs 1 for bf16. All DRAM loads are gpsimd casting DMAs
# (f32 -> bf16 on the fly). Attention processes 2 heads per matmul (they
# share the kv head) to halve instruction count and keep the PE pipelined.
import sys

import numpy as np

try:
    import concourse.bass as bass  # noqa: F401
except ImportError:
    for p in ("/opt/trn_rl_repo", "/root/.axon_site/_ro/trn_rl_repo"):
        if p not in sys.path:
            sys.path.append(p)
    import concourse.bass as bass  # noqa: F401

import concourse.bacc as bacc
import concourse.mybir as mybir
import concourse.tile as tile
from concourse import masks
from concourse.bass_utils import run_bass_kernel_spmd

S = 1024
D = 2048
H = 32
KV = 4
HD = 64
NH = 8  # q heads per core
P = 128
SC = S // P  # 8 seq chunks
DC = D // P  # 16 D chunks
N_CORES = 8
GROUPS = [[0, 1, 2, 3], [4, 5, 6, 7]]

F32 = mybir.dt.float32
BF16 = mybir.dt.bfloat16

_NC_CACHE = {}


def _build_graph():
    nc = bacc.Bacc("TRN2", target_bir_lowering=False, debug=False, num_devices=N_CORES)

    x_p = nc.dram_tensor("x", [S, D], F32, kind="ExternalInput")
    wq_p = nc.dram_tensor("wq", [D, NH * HD], F32, kind="ExternalInput")
    wkv_p = nc.dram_tensor("wkv", [D, 2 * HD], F32, kind="ExternalInput")
    wo_p = nc.dram_tensor("wo", [NH * HD, D], F32, kind="ExternalInput")
    cs_p = nc.dram_tensor("cs8", [S, NH * 32], F32, kind="ExternalInput")
    sn_p = nc.dram_tensor("sn8", [S, NH * 32], F32, kind="ExternalInput")
    mk_p = nc.dram_tensor("mk8", [SC, P, P], F32, kind="ExternalInput")
    out_p = nc.dram_tensor("out", [S // 4, D], F32, kind="ExternalOutput")

    with tile.TileContext(nc) as tc:
        with (
            tc.tile_pool(name="const", bufs=1) as constp,
            tc.tile_pool(name="big", bufs=1) as bigp,
            tc.tile_pool(name="work", bufs=1) as workp,
            tc.tile_pool(name="rt", bufs=4) as rtp,
            tc.tile_pool(name="attn", bufs=3) as attnp,
            tc.tile_pool(name="opart", bufs=2) as opartp,
            tc.tile_pool(name="tiny", bufs=4) as tinyp,
            tc.tile_pool(name="psum", bufs=1, space="PSUM") as psump,
            tc.tile_pool(name="dram", bufs=1, space="DRAM") as dramp,
        ):
            # ---- constants (cheap engine work first; DMAs ordered by need) ----
            ident = constp.tile([P, P], BF16, tag="ident")
            masks.make_identity(nc, ident[:])
            ones64 = constp.tile([1, 64], BF16, tag="ones64")
            nc.any.memset(ones64[:], 1.0)

            # x loads split per s-chunk so transposes can start early
            xt_all = bigp.tile([P, SC * D], BF16, tag="x_all")  # x[128s+p, d] at (p, s, d)

            def load_x(s):
                nc.gpsimd.dma_start(
                    out=xt_all[:, s * D : (s + 1) * D],
                    in_=x_p[s * P : (s + 1) * P, :],
                )

            load_x(0)
            wkv_all = bigp.tile([P, DC * 2 * HD], BF16, tag="wkv_all")
            nc.gpsimd.dma_start(
                out=wkv_all[:].rearrange("p (c n) -> p c n", c=DC),
                in_=wkv_p[:, :].rearrange("(c p) n -> p c n", p=P),
            )
            load_x(1)
            wq_all = bigp.tile([P, DC * 512], BF16, tag="wq_all")
            nc.gpsimd.dma_start(
                out=wq_all[:].rearrange("p (c n) -> p c n", c=DC),
                in_=wq_p[:, :].rearrange("(c p) n -> p c n", p=P),
            )
            for s in range(2, SC):
                load_x(s)
            cst = constp.tile([P, SC * 256], F32, tag="cst")  # cos, tiled x8 heads
            nc.gpsimd.dma_start(
                out=cst[:].rearrange("p (s c) -> p s c", s=SC),
                in_=cs_p[:, :].rearrange("(s p) c -> p s c", p=P),
            )
            snt = constp.tile([P, SC * 256], F32, tag="snt")
            nc.gpsimd.dma_start(
                out=snt[:].rearrange("p (s c) -> p s c", s=SC),
                in_=sn_p[:, :].rearrange("(s p) c -> p s c", p=P),
            )
            mkt = constp.tile([P, SC * P], F32, tag="mkt")  # 8 transposed diag blocks *8
            nc.gpsimd.dma_start(
                out=mkt[:].rearrange("p (j c) -> p j c", j=SC),
                in_=mk_p[:, :, :].rearrange("j p c -> p j c"),
            )
            wo_all = bigp.tile([P, 4 * D], BF16, tag="wo_all")
            nc.gpsimd.dma_start(
                out=wo_all[:].rearrange("p (c n) -> p c n", c=4),
                in_=wo_p[:, :].rearrange("(c p) n -> p c n", p=P),
            )

            # ---- xT: transpose x to [D, S] layout (bf16, via PE) ----
            xT = [workp.tile([P, S], BF16, tag=f"xT{d}", name=f"xT{d}") for d in range(DC)]
            for s in range(SC):
                for d in range(DC):
                    tp = psump.tile([P, P], BF16, tag="sc", bufs=3)
                    nc.tensor.transpose(
                        tp[:], xt_all[:, s * D + d * P : s * D + (d + 1) * P], ident[:]
                    )
                    nc.vector.tensor_copy(xT[d][:, s * P : (s + 1) * P], tp[:])

            # ---- projections + RoPE (natural [s, ch] layout) ----
            qrot = [workp.tile([P, NH * HD], BF16, tag=f"qr{s}", name=f"qr{s}") for s in range(SC)]
            krot = [workp.tile([P, HD], BF16, tag=f"kr{s}", name=f"kr{s}") for s in range(SC)]
            vaug = [workp.tile([P, HD + 1], BF16, tag=f"va{s}", name=f"va{s}") for s in range(SC)]

            def rope(ps_ap, dst, s, nh):
                # ps_ap: PSUM AP [128, nh*64] f32; dst: SBUF bf16 same shape
                pv = ps_ap.rearrange("p (h t c) -> p h t c", h=nh, t=2)
                dv = dst[:].rearrange("p (h t c) -> p h t c", h=nh, t=2)
                cs = cst[:, s * 256 : s * 256 + nh * 32].rearrange(
                    "p (h c) -> p h c", h=nh
                )
                sn = snt[:, s * 256 : s * 256 + nh * 32].rearrange(
                    "p (h c) -> p h c", h=nh
                )
                lo, hi = pv[:, :, 0, :], pv[:, :, 1, :]
                t1 = rtp.tile([P, NH * 32], F32, tag="rt1")
                t2 = rtp.tile([P, NH * 32], F32, tag="rt2")
                t1v = t1[:, : nh * 32].rearrange("p (h c) -> p h c", h=nh)
                t2v = t2[:, : nh * 32].rearrange("p (h c) -> p h c", h=nh)
                nc.any.tensor_mul(t1v, lo, cs)
                nc.any.tensor_mul(t2v, hi, sn)
                nc.any.tensor_sub(dv[:, :, 0, :], t1v, t2v)
                nc.any.tensor_mul(t1v, hi, cs)
                nc.any.tensor_mul(t2v, lo, sn)
                nc.any.tensor_add(dv[:, :, 1, :], t1v, t2v)

            for s in range(SC):
                pkv = psump.tile([P, 2 * HD], F32, tag="mm512", bufs=3)
                for d in range(DC):
                    nc.tensor.matmul(
                        pkv[:],
                        xT[d][:, s * P : (s + 1) * P],
                        wkv_all[:, d * 2 * HD : (d + 1) * 2 * HD],
                        start=(d == 0),
                        stop=(d == DC - 1),
                    )
                rope(pkv[:, 0:HD], krot[s], s, 1)
                nc.vector.tensor_copy(vaug[s][:, 0:HD], pkv[:, HD : 2 * HD])
                nc.any.memset(vaug[s][:, HD : HD + 1], 1.0)

                pq = psump.tile([P, NH * HD], F32, tag="mm512", bufs=3)
                for d in range(DC):
                    nc.tensor.matmul(
                        pq[:],
                        xT[d][:, s * P : (s + 1) * P],
                        wq_all[:, d * 512 : (d + 1) * 512],
                        start=(d == 0),
                        stop=(d == DC - 1),
                    )
                rope(pq[:, :], qrot[s], s, NH)

            # ---- per-head transposes into one combined qT tile + kT ----
            # qTall columns: h*1024 + c*512 + a*128 + i  (s-chunk = 4c + a)
            qTall = workp.tile([64, NH * S], BF16, tag="qTall")
            kT = workp.tile([64, S], BF16, tag="kT")
            for s in range(SC):
                tpk = psump.tile([64, P], BF16, tag="sc", bufs=3)
                nc.tensor.transpose(tpk[:], krot[s][:], ident[:])
                nc.vector.tensor_copy(kT[:, s * P : (s + 1) * P], tpk[:])
                for h in range(NH):
                    tpq = psump.tile([64, P], BF16, tag="sc", bufs=3)
                    nc.tensor.transpose(
                        tpq[:], qrot[s][:, h * HD : (h + 1) * HD], ident[:]
                    )
                    nc.vector.tensor_copy(
                        qTall[:, h * S + s * P : h * S + (s + 1) * P], tpq[:]
                    )

            # view: [64, a(4), c(2), h(8), i(128)]; a scores matmul's rhs takes
            # (c, h, i)-ordered columns: [j:h | j:h' | j+4:h | j+4:h']
            qview = qTall[:].rearrange("p (h c a b) -> p a c h b", h=NH, c=2, a=4, b=P)

            def _normalize(j, oa, hp):
                # oa: PSUM [65, 512]; row 64 = softmax denominators
                rec = tinyp.tile([1, 512], F32, tag="rec")
                nc.vector.reciprocal(rec[:], oa[HD : HD + 1, :])
                recb = tinyp.tile([1, 512], BF16, tag="recb")
                nc.vector.tensor_copy(recb[:], rec[:])
                rb = psump.tile([64, 512], F32, tag="sc", bufs=3)
                nc.tensor.matmul(rb[:], ones64[:], recb[:], start=True, stop=True)
                rb_sb = tinyp.tile([64, 512], F32, tag="rbs")
                nc.vector.tensor_copy(rb_sb[:], rb[:])
                for oi, qb in ((0, j), (1, j + 4)):
                    for hh in range(2):
                        nc.vector.tensor_mul(
                            oT[hp][hh * 64 : (hh + 1) * 64, qb * P : (qb + 1) * P],
                            oa[0:HD, oi * 256 + hh * P : oi * 256 + (hh + 1) * P],
                            rb_sb[:, oi * 256 + hh * P : oi * 256 + (hh + 1) * P],
                        )

            # ---- attention (2 heads per matmul) + per-pair o_proj and RS ----
            oT = [workp.tile([P, S], BF16, tag=f"oT{c}", name=f"oT{c}") for c in range(4)]
            rs_in = [dramp.tile([256, D], BF16, name=f"rsin{j}") for j in range(4)]
            rs_out = [dramp.tile([64, D], BF16, name=f"rsout{j}") for j in range(4)]

            for j in range(4):  # q-block pair (j, j+4)
                pending = None  # deferred normalization: (oa, hp)
                for hp in range(NH // 2):  # head pair (2hp, 2hp+1)
                    # phase 1: stream all score matmuls + exp into SBUF
                    ats = []
                    for skc in range(j + 5):
                        both = skc <= j
                        ncols = 512 if both else 256
                        sc_t = psump.tile([P, ncols], F32, tag="sc", bufs=3)
                        at_t = attnp.tile([P, ncols], BF16, tag="at", bufs=9)
                        if both:
                            nc.tensor.matmul(
                                sc_t[:],
                                kT[:, skc * P : (skc + 1) * P],
                                qview[:, j, :, 2 * hp : 2 * hp + 2, :],
                                start=True,
                                stop=True,
                            )
                        else:
                            nc.tensor.matmul(
                                sc_t[:],
                                kT[:, skc * P : (skc + 1) * P],
                                qview[:, j, 1, 2 * hp : 2 * hp + 2, :],
                                start=True,
                                stop=True,
                            )
                        if skc == j:
                            for hh in range(2):
                                nc.vector.tensor_add(
                                    sc_t[:, hh * P : (hh + 1) * P],
                                    sc_t[:, hh * P : (hh + 1) * P],
                                    mkt[:, j * P : (j + 1) * P],
                                )
                        if skc == j + 4:
                            off = ncols - 256
                            for hh in range(2):
                                nc.vector.tensor_add(
                                    sc_t[:, off + hh * P : off + (hh + 1) * P],
                                    sc_t[:, off + hh * P : off + (hh + 1) * P],
                                    mkt[:, (j + 4) * P : (j + 5) * P],
                                )
                        nc.scalar.activation(
                            at_t[:],
                            sc_t[:],
                            mybir.ActivationFunctionType.Exp,
                            scale=0.125,
                        )
                        ats.append((at_t, ncols))
                    # phase 2: dense attnv accumulation runs into one PSUM bank
                    # (cols 0:256 = q-block j, 256:512 = q-block j+4)
                    oa = psump.tile([HD + 1, 512], F32, tag="oa", bufs=2)
                    for skc in range(j + 1):
                        at_t, ncols = ats[skc]
                        nc.tensor.matmul(
                            oa[:, 0:256],
                            vaug[skc][:],
                            at_t[:, 0:256],
                            start=(skc == 0),
                            stop=(skc == j),
                            skip_group_check=True,
                        )
                    for skc in range(j + 5):
                        at_t, ncols = ats[skc]
                        nc.tensor.matmul(
                            oa[:, 256:512],
                            vaug[skc][:],
                            at_t[:, (ncols - 256) : ncols],
                            start=(skc == 0),
                            stop=(skc == j + 4),
                            skip_group_check=True,
                        )
                    if pending is not None:
                        _normalize(j, *pending)
                    pending = (oa, hp)
                if pending is not None:
                    _normalize(j, *pending)
                # o_proj for the two finished s-chunks (j and j+4)
                for si, s in enumerate((j, j + 4)):
                    op_t = opartp.tile([P, D], BF16, tag="op")
                    for n in range(4):
                        po = psump.tile([P, 512], F32, tag="mm512", bufs=3)
                        for c in range(4):
                            nc.tensor.matmul(
                                po[:],
                                oT[c][:, s * P : (s + 1) * P],
                                wo_all[:, c * D + n * 512 : c * D + (n + 1) * 512],
                                start=(c == 0),
                                stop=(c == 3),
                            )
                        nc.vector.tensor_copy(op_t[:, n * 512 : (n + 1) * 512], po[:])
                    nc.sync.dma_start(
                        out=rs_in[j][si * P : (si + 1) * P, :], in_=op_t[:]
                    )
                nc.gpsimd.collective_compute(
                    "ReduceScatter",
                    mybir.AluOpType.add,
                    replica_groups=GROUPS,
                    ins=[rs_in[j].opt()],
                    outs=[rs_out[j].opt()],
                )
                # final cast bf16 -> f32, DRAM -> DRAM
                nc.gpsimd.dma_start(
                    out=out_p[j * 64 : (j + 1) * 64, :], in_=rs_out[j][:, :]
                )

    nc.compile()
    return nc


def _get_nc():
    if "nc" not in _NC_CACHE:
        _NC_CACHE["nc"] = _build_graph()
    return _NC_CACHE["nc"]


def _shard_inputs(x, wq, wk, wv, wo, cos, sin, mask, pos):
    x = np.asarray(x, dtype=np.float32)
    wq = np.asarray(wq, dtype=np.float32)
    wk = np.asarray(wk, dtype=np.float32)
    wv = np.asarray(wv, dtype=np.float32)
    wo = np.asarray(wo, dtype=np.float32)
    cos = np.asarray(cos, dtype=np.float32)
    sin = np.asarray(sin, dtype=np.float32)
    mask = np.asarray(mask, dtype=np.float32)
    p = int(pos)

    cs = cos[p : p + S]  # [S, 32]
    sn = sin[p : p + S]
    cs8 = np.ascontiguousarray(np.tile(cs, (1, NH)))  # [S, 256]
    sn8 = np.ascontiguousarray(np.tile(sn, (1, NH)))
    # transposed diagonal 128x128 blocks of the additive mask, pre-scaled by
    # sqrt(HD) so exp(scale*(scores + 8*mask)) == exp(scores/8 + mask)
    mk8 = np.stack(
        [8.0 * mask[j * P : (j + 1) * P, j * P : (j + 1) * P].T for j in range(SC)]
    ).astype(np.float32)
    mk8 = np.ascontiguousarray(mk8)

    in_maps = []
    for d in range(N_CORES):
        g, kv = d // 4, d % 4
        in_maps.append(
            {
                "x": np.ascontiguousarray(x[g]),
                "wq": np.ascontiguousarray(wq[:, kv * 512 : (kv + 1) * 512]),
                "wkv": np.ascontiguousarray(
                    np.concatenate(
                        [
                            wk[:, kv * HD : (kv + 1) * HD],
                            wv[:, kv * HD : (kv + 1) * HD],
                        ],
                        axis=1,
                    )
                ),
                "wo": np.ascontiguousarray(wo[kv * 512 : (kv + 1) * 512, :]),
                "cs8": cs8,
                "sn8": sn8,
                "mk8": mk8,
            }
        )
    return in_maps


def _run(inputs, trace=False, trace_kwargs=None):
    nc = _get_nc()
    in_maps = _shard_inputs(**inputs)
    res = run_bass_kernel_spmd(
        nc,
        in_maps,
        core_ids=list(range(N_CORES)),
        trace=trace,
        **(trace_kwargs or {}),
    )
    B = 2
    out = np.empty((B, S, D), dtype=np.float32)
    for d in range(N_CORES):
        g, r = d // 4, d % 4
        core_out = res.results[d]["out"]  # [256, D]; rows 64j.. = pair-j shard r
        for j in range(4):
            blk = core_out[j * 64 : (j + 1) * 64, :]
            if r < 2:
                out[g, j * P + r * 64 : j * P + (r + 1) * 64, :] = blk
            else:
                out[g, (j + 4) * P + (r - 2) * 64 : (j + 4) * P + (r - 1) * 64, :] = blk
    return out, res


def kernel(**inputs) -> np.ndarray:
    out, _ = _run(inputs, trace=False)
    return out
